# revision 14
# baseline (speedup 1.0000x reference)
"""Attention-with-dropout kernel for 8 Trainium2 NeuronCores.

Reference math (fp32, per batch b / head h):
    scores = (x1 @ x2.T) * 10000
    probs  = softmax(scores, axis=-1)
    keep   = jax.random.bernoulli(key(42), 0.8, probs.shape)
    probs  = where(keep, probs / 0.8, 0)
    out    = probs @ x2

Sharding: the 32 (batch, head) pairs are split across 8 cores, 4 heads per
core; no cross-core communication.

Per-core kernel design:
  * scores are produced on the tensor engine in fp16 hi/lo split form
    (x*100 -> hi + lo fp16); hi*hi + lo*hi via one K=128 stacked matmul,
    hi*lo via a row-group-packed pair of K=64 matmuls.  Error vs fp32 is
    ~4e-6 on scores with std ~8, so the softmax argmax matches fp32.
  * row max via DVE reduce over PSUM, exp via ScalarE with fused
    bias=-max and accum_out giving the softmax denominator for free.
  * dropout mask is the exact jax PRNG mask, precomputed host-side once,
    shipped as fp16 {0, 1.25} and applied on GPSIMD.
  * masked probs are transposed 128x128 on the tensor engine so the PV
    matmul can contract over k; normalization by 1/sum is folded into the
    final PSUM->SBUF copy (tensor_scalar per-partition multiply).
"""

import functools

import numpy as np

B, H, S, D = 2, 16, 2048, 64
BH = B * H
N_CORES = 8
HPC = BH // N_CORES  # heads per core
SIDE = 100.0  # per-side input scale; SIDE**2 == reference SCALE (10000)


def _hi(a):
    return a.astype(np.float16)


def _lo(a):
    return (a - a.astype(np.float16).astype(np.float32)).astype(np.float16)


@functools.cache
def _mask_f16():
    """Dropout keep-mask, pre-scaled: {0, 1.25} fp16, shape [BH, S, S]."""
    import jax

    cpu = jax.devices("cpu")[0]
    with jax.default_device(cpu):
        keep = jax.random.bernoulli(jax.random.key(42), 0.8, (B, H, S, S))
        keep = np.asarray(keep)
    m = np.where(keep, np.float16(1.25), np.float16(0.0))
    return m.reshape(BH, S, S)


@functools.cache
def _program(n_heads=HPC, sq=S, sk=S):
    """Build the per-core Bass/Tile program (SPMD: same program, 8 cores)."""
    from contextlib import ExitStack

    import concourse.bass as bass
    import concourse.tile as tile
    from concourse import mybir
    from concourse.masks import make_identity

    f16 = mybir.dt.float16
    f32 = mybir.dt.float32
    Act = mybir.ActivationFunctionType

    nqt = sq // 128  # q tiles per head
    nkc = sk // 128  # k chunks of 128
    nsc = sk // 1024  # score PSUM tiles ([128,1024]) per q tile
    half = sk // 1024  # 512-chunk offset of the packed pass-2 B member
    ngr = nqt // 4  # q groups of 512
    assert sq % 512 == 0 and sk % 1024 == 0

    def _split_excess_waits(max_waits=1):
        """Walrus encodes at most ~2 sync commands per compute instruction.
        Hoist excess waits onto no-op instructions inserted just before, on
        the same engine queue (identical blocking semantics, staged)."""
        skip = (mybir.InstEventSemaphore, mybir.InstNoOp)
        n_fix = 0
        for fn in nc.m.functions:
            for blk in fn.blocks:
                out = []
                for inst in blk.instructions:
                    si = inst.sync_info
                    if (
                        si is not None
                        and si.on_wait
                        and len(si.on_wait) > max_waits
                        and not isinstance(inst, skip)
                    ):
                        waits = list(si.on_wait)
                        for w in waits[:-1]:
                            nop = mybir.InstNoOp(
                                name=f"I-waitfix-{n_fix}",
                                engine=inst.engine,
                                sync_info=mybir.SyncInfo(on_wait=[w], on_update=[]),
                                bass_nofuse=True,
                            )
                            out.append(nop)
                            n_fix += 1
                        inst.sync_info = mybir.SyncInfo(
                            on_wait=waits[-1:], on_update=list(si.on_update or [])
                        )
                    out.append(inst)
                blk.instructions = out
        return n_fix

    nc = bass.Bass()
    p_lhs1 = nc.declare_dram_parameter("qk_lhs1", [n_heads, 128, sq], f16, False)
    p_lhs2 = nc.declare_dram_parameter("qk_lhs2", [n_heads, 128, sq], f16, False)
    p_rhs1 = nc.declare_dram_parameter("qk_rhs1", [n_heads, 128, sk], f16, False)
    p_rhs2 = nc.declare_dram_parameter("qk_rhs2", [n_heads, 128, sk // 2], f16, False)
    p_pvw = nc.declare_dram_parameter("pv_w", [n_heads, 128, nkc * 64], f16, False)
    p_mask = nc.declare_dram_parameter("mask", [n_heads, sq, sk], f16, False)
    p_out = nc.declare_dram_parameter("out", [n_heads, sq, 64], f32, True)

    with tile.TileContext(nc) as tc, ExitStack() as ctx:
        const = ctx.enter_context(tc.tile_pool(name="const", bufs=1))
        heads = ctx.enter_context(tc.tile_pool(name="heads", bufs=2))
        masks = ctx.enter_context(tc.tile_pool(name="masks", bufs=5))
        probs = ctx.enter_context(tc.tile_pool(name="probs", bufs=4))
        pmts = ctx.enter_context(tc.tile_pool(name="pmts", bufs=2))
        stats = ctx.enter_context(tc.tile_pool(name="stats", bufs=32))
        outs = ctx.enter_context(tc.tile_pool(name="outs", bufs=8))
        # PSUM: "scps" slots are 2 banks each (scores / PV / out-transpose
        # share the tag); "tpps" holds fp16 transpose targets, 1 bank each.
        scps = ctx.enter_context(tc.tile_pool(name="scps", bufs=3, space="PSUM"))
        tpps = ctx.enter_context(tc.tile_pool(name="tpps", bufs=2, space="PSUM"))

        id16 = const.tile([128, 128], f16)
        make_identity(nc, id16)
        id32 = const.tile([64, 64], f32)
        make_identity(nc, id32)

        for h in range(n_heads):
            lhs1 = heads.tile([128, sq], f16, tag="lhs1")
            nc.sync.dma_start(out=lhs1, in_=p_lhs1[h])
            lhs2 = heads.tile([128, sq], f16, tag="lhs2")
            nc.sync.dma_start(out=lhs2, in_=p_lhs2[h])
            rhs1 = heads.tile([128, sk], f16, tag="rhs1")
            nc.sync.dma_start(out=rhs1, in_=p_rhs1[h])
            rhs2 = heads.tile([128, sk // 2], f16, tag="rhs2")
            nc.sync.dma_start(out=rhs2, in_=p_rhs2[h])
            pvw = heads.tile([128, nkc * 64], f16, tag="pvw")
            nc.sync.dma_start(out=pvw, in_=p_pvw[h])

            for g in range(ngr):
                pmt = pmts.tile([128, nkc * 512], f16, tag="pmt")
                pmt3 = pmt.rearrange("p (j q) -> p j q", j=nkc)
                rzs = []
                for tt in range(4):
                    t = g * 4 + tt
                    qs = bass.ts(t, 128)

                    # ---- QK^T: scores*1e4 into PSUM (fp32) ----
                    sc = [
                        scps.tile([128, 1024], f32, tag="sc", name=f"sc{m}")
                        for m in range(nsc)
                    ]
                    for n in range(sk // 512):
                        nc.tensor.matmul(
                            sc[n // 2][:, bass.ts(n % 2, 512)],
                            lhsT=lhs1[:, qs],
                            rhs=rhs1[:, bass.ts(n, 512)],
                            start=True,
                            stop=False,
                        )
                    for i in range(half):
                        nA, nB = i, i + half
                        nc.tensor.matmul(
                            sc[nA // 2][:, bass.ts(nA % 2, 512)],
                            lhsT=lhs2[0:64, qs],
                            rhs=rhs2[0:64, bass.ts(i, 512)],
                            start=False,
                            stop=True,
                            tile_position=(0, 0),
                        )
                        nc.tensor.matmul(
                            sc[nB // 2][:, bass.ts(nB % 2, 512)],
                            lhsT=lhs2[64:128, qs],
                            rhs=rhs2[64:128, bass.ts(i, 512)],
                            start=False,
                            stop=True,
                            tile_position=(64, 0),
                        )

                    # ---- softmax stats + exp ----
                    # Tiny ScalarE read of the last-written score slice: absorbs
                    # the PE wait onto ACT's observed tick so the exp below
                    # needs only {DVE, self} waits (walrus AC limit is 2).
                    probe = stats.tile([128, 1], f32, tag="probe")
                    nc.scalar.copy(probe, sc[-1][:, 1023:1024])

                    mx = stats.tile([128, nsc], f32, tag="mx")
                    for m in range(nsc):
                        nc.vector.reduce_max(
                            mx[:, m : m + 1],
                            sc[m],
                            axis=mybir.AxisListType.X,
                            negate=True,
                        )
                    # negmx = -max over chunks = min of per-chunk negated maxes.
                    # On GPSIMD so the exp's bias/WAR deps stay on one engine.
                    negmx = stats.tile([128, 1], f32, tag="negmx")
                    if nsc == 1:
                        nc.vector.tensor_copy(negmx, mx)
                    else:
                        nc.vector.tensor_tensor(
                            negmx, mx[:, 0:1], mx[:, 1:2], op=mybir.AluOpType.min
                        )
                    p = probs.tile([128, sk], f16, tag="p")
                    zp = stats.tile([128, nsc], f32, tag="zp")
                    for m in range(nsc):
                        nc.scalar.activation(
                            p[:, bass.ts(m, 1024)],
                            sc[m],
                            Act.Exp,
                            bias=negmx,
                            scale=1.0,
                            accum_out=zp[:, m : m + 1],
                        )
                    z = stats.tile([128, 1], f32, tag="z")
                    if nsc == 1:
                        nc.vector.tensor_copy(z, zp)
                    else:
                        nc.vector.tensor_add(z, zp[:, 0:1], zp[:, 1:2])
                    rz = stats.tile([128, 1], f32, tag="rz")
                    nc.vector.reciprocal(rz, z)
                    rzs.append(rz)

                    # ---- dropout mask (exact PRNG, prescaled {0,1.25}) ----
                    mk = masks.tile([128, sk], f16, tag="mk")
                    nc.sync.dma_start(out=mk, in_=p_mask[h, qs, :])
                    pm = probs.tile([128, sk], f16, tag="pm")
                    nc.vector.tensor_mul(pm, p, mk)

                    # ---- transpose masked probs: 128x128 blocks -> pmt ----
                    for c in range(nkc // 8):
                        tp = tpps.tile([128, 1024], f16, tag="tp")
                        for bb in range(8):
                            j = c * 8 + bb
                            nc.tensor.transpose(
                                tp[:, bass.ts(bb, 128)],
                                pm[:, bass.ts(j, 128)],
                                id16,
                            )
                        src = tp.rearrange("p (b q) -> p b q", b=8)
                        dst = pmt3[:, c * 8 : (c + 1) * 8, bass.ts(tt, 128)]
                        nc.scalar.copy(dst, src)

                # ---- PV: out.T[d, q] accumulated over k chunks ----
                pv = scps.tile([64, 512], f32, tag="sc")
                for j in range(nkc):
                    nc.tensor.matmul(
                        pv,
                        lhsT=pvw[:, bass.ts(j, 64)],
                        rhs=pmt3[:, j, :],
                        start=(j == 0),
                        stop=(j == nkc - 1),
                    )
                pvs = outs.tile([64, 512], f32, tag="pvs")
                nc.vector.tensor_copy(pvs, pv)

                # ---- transpose back to [q, d], normalize by 1/sum, store ----
                for tt in range(4):
                    t = g * 4 + tt
                    ot = scps.tile([128, 64], f32, tag="sc")
                    nc.tensor.transpose(ot, pvs[:, bass.ts(tt, 128)], id32)
                    os_ = outs.tile([128, 64], f32, tag="os")
                    nc.vector.tensor_scalar_mul(os_, ot, rzs[tt])
                    nc.sync.dma_start(out=p_out[h, bass.ts(t, 128), :], in_=os_)

    _split_excess_waits()
    return nc


def _host_inputs(x1, x2, n_heads_total=BH, sq=S, sk=S):
    """Prepare all DRAM tensors in SBUF-friendly layouts, fp32 -> fp16 hi/lo."""
    nkc = sk // 128
    x1s = (x1.astype(np.float32) * SIDE).reshape(n_heads_total, sq, D)
    x2s = (x2.astype(np.float32) * SIDE).reshape(n_heads_total, sk, D)

    x1t = np.ascontiguousarray(x1s.transpose(0, 2, 1))  # [BH, 64, SQ]
    x2t = np.ascontiguousarray(x2s.transpose(0, 2, 1))  # [BH, 64, SK]

    x1t_hi, x1t_lo = _hi(x1t), _lo(x1t)
    x2t_hi, x2t_lo = _hi(x2t), _lo(x2t)

    qk_lhs1 = np.concatenate([x1t_hi, x1t_lo], axis=1)  # [BH,128,SQ]
    qk_lhs2 = np.concatenate([x1t_hi, x1t_hi], axis=1)  # [BH,128,SQ]
    qk_rhs1 = np.concatenate([x2t_hi, x2t_hi], axis=1)  # [BH,128,SK]
    qk_rhs2 = np.concatenate(
        [x2t_lo[:, :, : sk // 2], x2t_lo[:, :, sk // 2 :]], axis=1
    )  # [BH,128,SK/2]

    # PV weights: unscaled x2, chunked [BH, 128, nkc*64]
    x2n = x2.astype(np.float32).reshape(n_heads_total, nkc, 128, D)
    pv_w = np.ascontiguousarray(x2n.transpose(0, 2, 1, 3)).reshape(
        n_heads_total, 128, nkc * D
    )
    pv_w = pv_w.astype(np.float16)
    return qk_lhs1, qk_lhs2, qk_rhs1, qk_rhs2, pv_w


def kernel(x1: np.ndarray, x2: np.ndarray) -> np.ndarray:
    from concourse.bass_utils import run_bass_kernel_spmd

    x1 = np.asarray(x1)
    x2 = np.asarray(x2)
    qk_lhs1, qk_lhs2, qk_rhs1, qk_rhs2, pv_w = _host_inputs(x1, x2)
    mask = _mask_f16()

    nc = _program()
    core_ids = list(range(N_CORES))
    in_maps = []
    for c in core_ids:
        sl = slice(c * HPC, (c + 1) * HPC)
        in_maps.append(
            {
                "qk_lhs1": qk_lhs1[sl],
                "qk_lhs2": qk_lhs2[sl],
                "qk_rhs1": qk_rhs1[sl],
                "qk_rhs2": qk_rhs2[sl],
                "pv_w": pv_w[sl],
                "mask": mask[sl],
            }
        )
    res = run_bass_kernel_spmd(nc, in_maps, core_ids)
    out = np.concatenate([r["out"] for r in res.results], axis=0)  # [BH,S,64]
    return out.reshape(B, H, S, D).astype(np.float32)


# revision 16
# speedup vs baseline: 20.5698x; 20.5698x over previous
"""Attention-with-dropout kernel for 8 Trainium2 NeuronCores.

Reference math (fp32, per batch b / head h):
    scores = (x1 @ x2.T) * 10000
    probs  = softmax(scores, axis=-1)
    keep   = jax.random.bernoulli(key(42), 0.8, probs.shape)
    probs  = where(keep, probs / 0.8, 0)
    out    = probs @ x2

Sharding: the 32 (batch, head) pairs are split across 8 cores, 4 heads per
core; no cross-core communication.

Per-core kernel design:
  * scores are produced on the tensor engine in fp16 hi/lo split form
    (x*100 -> hi + lo fp16); hi*hi + lo*hi via one K=128 stacked matmul,
    hi*lo via a row-group-packed pair of K=64 matmuls.  Error vs fp32 is
    ~4e-6 on scores with std ~8, so the softmax argmax matches fp32.
  * row max via DVE reduce over PSUM, exp via ScalarE with fused
    bias=-max and accum_out giving the softmax denominator for free.
  * dropout mask is the exact jax PRNG mask, precomputed host-side once,
    shipped as fp16 {0, 1.25} and applied on GPSIMD.
  * masked probs are transposed 128x128 on the tensor engine so the PV
    matmul can contract over k; normalization by 1/sum is folded into the
    final PSUM->SBUF copy (tensor_scalar per-partition multiply).
"""

import functools

import numpy as np

B, H, S, D = 2, 16, 2048, 64
BH = B * H
N_CORES = 8
HPC = BH // N_CORES  # heads per core
SIDE = 100.0  # per-side input scale; SIDE**2 == reference SCALE (10000)


def _hi(a):
    return a.astype(np.float16)


def _lo(a):
    return (a - a.astype(np.float16).astype(np.float32)).astype(np.float16)


@functools.cache
def _mask_f16():
    """Dropout keep-mask, pre-scaled: {0, 1.25} fp16, shape [BH, S, S]."""
    import jax

    cpu = jax.devices("cpu")[0]
    with jax.default_device(cpu):
        keep = jax.random.bernoulli(jax.random.key(42), 0.8, (B, H, S, S))
        keep = np.asarray(keep)
    m = np.where(keep, np.float16(1.25), np.float16(0.0))
    return m.reshape(BH, S, S)


def _split_excess_waits(nc, mybir, max_waits=1):
    """Walrus encodes at most ~2 sync commands per instruction.  Hoist
    excess waits onto no-op instructions inserted just before, on the same
    engine queue (identical blocking semantics, staged)."""
    skip = (mybir.InstEventSemaphore, mybir.InstNoOp)
    n_fix = 0
    for fn in nc.m.functions:
        for blk in fn.blocks:
            out = []
            for inst in blk.instructions:
                si = inst.sync_info
                if (
                    si is not None
                    and si.on_wait
                    and len(si.on_wait) > max_waits
                    and not isinstance(inst, skip)
                ):
                    waits = list(si.on_wait)
                    for w in waits[:-1]:
                        nop = mybir.InstNoOp(
                            name=f"I-waitfix-{n_fix}",
                            engine=inst.engine,
                            sync_info=mybir.SyncInfo(on_wait=[w], on_update=[]),
                            bass_nofuse=True,
                        )
                        out.append(nop)
                        n_fix += 1
                    inst.sync_info = mybir.SyncInfo(
                        on_wait=waits[-1:], on_update=list(si.on_update or [])
                    )
                out.append(inst)
            blk.instructions = out
    return n_fix


@functools.cache
def _program(n_heads=HPC, sq=S, sk=S, fix_waits=True):
    """Build the per-core Bass/Tile program (SPMD: same program, 8 cores)."""
    from contextlib import ExitStack

    import concourse.bass as bass
    import concourse.tile as tile
    from concourse import mybir
    from concourse.masks import make_identity

    f16 = mybir.dt.float16
    f32 = mybir.dt.float32
    Act = mybir.ActivationFunctionType

    nqt = sq // 128  # q tiles per head
    nkc = sk // 128  # k chunks of 128
    nsc = sk // 1024  # score PSUM tiles ([128,1024]) per q tile
    half = sk // 1024  # 512-chunk offset of the packed pass-2 B member
    ngr = nqt // 4  # q groups of 512
    assert sq % 512 == 0 and sk % 1024 == 0

    nc = bass.Bass()
    p_lhs1 = nc.declare_dram_parameter("qk_lhs1", [n_heads, 128, sq], f16, False)
    p_lhs2 = nc.declare_dram_parameter("qk_lhs2", [n_heads, 128, sq], f16, False)
    p_rhs1 = nc.declare_dram_parameter("qk_rhs1", [n_heads, 128, sk], f16, False)
    p_rhs2 = nc.declare_dram_parameter("qk_rhs2", [n_heads, 128, sk // 2], f16, False)
    p_pvw = nc.declare_dram_parameter("pv_w", [n_heads, 128, nkc * 64], f16, False)
    p_mask = nc.declare_dram_parameter("mask", [n_heads, sq, sk], f16, False)
    p_out = nc.declare_dram_parameter("out", [n_heads, sq, 64], f32, True)

    with tile.TileContext(nc) as tc, ExitStack() as ctx:
        const = ctx.enter_context(tc.tile_pool(name="const", bufs=1))
        heads = ctx.enter_context(tc.tile_pool(name="heads", bufs=2))
        masks = ctx.enter_context(tc.tile_pool(name="masks", bufs=5))
        probs = ctx.enter_context(tc.tile_pool(name="probs", bufs=4))
        pmts = ctx.enter_context(tc.tile_pool(name="pmts", bufs=2))
        stats = ctx.enter_context(tc.tile_pool(name="stats", bufs=32))
        outs = ctx.enter_context(tc.tile_pool(name="outs", bufs=8))
        # PSUM: "scps" slots are 2 banks each (scores / PV / out-transpose
        # share the tag); "tpps" holds fp16 transpose targets, 1 bank each.
        scps = ctx.enter_context(tc.tile_pool(name="scps", bufs=3, space="PSUM"))
        tpps = ctx.enter_context(tc.tile_pool(name="tpps", bufs=2, space="PSUM"))

        id16 = const.tile([128, 128], f16)
        make_identity(nc, id16)
        id32 = const.tile([64, 64], f32)
        make_identity(nc, id32)

        for h in range(n_heads):
            lhs1 = heads.tile([128, sq], f16, tag="lhs1")
            nc.sync.dma_start(out=lhs1, in_=p_lhs1[h])
            lhs2 = heads.tile([128, sq], f16, tag="lhs2")
            nc.sync.dma_start(out=lhs2, in_=p_lhs2[h])
            rhs1 = heads.tile([128, sk], f16, tag="rhs1")
            nc.sync.dma_start(out=rhs1, in_=p_rhs1[h])
            rhs2 = heads.tile([128, sk // 2], f16, tag="rhs2")
            nc.sync.dma_start(out=rhs2, in_=p_rhs2[h])
            pvw = heads.tile([128, nkc * 64], f16, tag="pvw")
            nc.sync.dma_start(out=pvw, in_=p_pvw[h])

            for g in range(ngr):
                pmt = pmts.tile([128, nkc * 512], f16, tag="pmt")
                pmt3 = pmt.rearrange("p (j q) -> p j q", j=nkc)
                rzs = []
                for tt in range(4):
                    t = g * 4 + tt
                    qs = bass.ts(t, 128)

                    # ---- QK^T: scores*1e4 into PSUM (fp32) ----
                    sc = [
                        scps.tile([128, 1024], f32, tag="sc", name=f"sc{m}")
                        for m in range(nsc)
                    ]
                    for n in range(sk // 512):
                        nc.tensor.matmul(
                            sc[n // 2][:, bass.ts(n % 2, 512)],
                            lhsT=lhs1[:, qs],
                            rhs=rhs1[:, bass.ts(n, 512)],
                            start=True,
                            stop=False,
                        )
                    for i in range(half):
                        nA, nB = i, i + half
                        nc.tensor.matmul(
                            sc[nA // 2][:, bass.ts(nA % 2, 512)],
                            lhsT=lhs2[0:64, qs],
                            rhs=rhs2[0:64, bass.ts(i, 512)],
                            start=False,
                            stop=True,
                            tile_position=(0, 0),
                        )
                        nc.tensor.matmul(
                            sc[nB // 2][:, bass.ts(nB % 2, 512)],
                            lhsT=lhs2[64:128, qs],
                            rhs=rhs2[64:128, bass.ts(i, 512)],
                            start=False,
                            stop=True,
                            tile_position=(64, 0),
                        )

                    # ---- softmax stats + exp ----
                    # Tiny ScalarE read of the last-written score slice: absorbs
                    # the PE wait onto ACT's observed tick so the exp below
                    # needs only {DVE, self} waits (walrus AC limit is 2).
                    probe = stats.tile([128, 1], f32, tag="probe")
                    nc.scalar.copy(probe, sc[-1][:, 1023:1024])

                    mx = stats.tile([128, nsc], f32, tag="mx")
                    for m in range(nsc):
                        nc.vector.reduce_max(
                            mx[:, m : m + 1],
                            sc[m],
                            axis=mybir.AxisListType.X,
                            negate=True,
                        )
                    # negmx = -max over chunks = min of per-chunk negated maxes.
                    # On GPSIMD so the exp's bias/WAR deps stay on one engine.
                    negmx = stats.tile([128, 1], f32, tag="negmx")
                    if nsc == 1:
                        nc.vector.tensor_copy(negmx, mx)
                    else:
                        nc.vector.tensor_tensor(
                            negmx, mx[:, 0:1], mx[:, 1:2], op=mybir.AluOpType.min
                        )
                    p = probs.tile([128, sk], f16, tag="p")
                    zp = stats.tile([128, nsc], f32, tag="zp")
                    for m in range(nsc):
                        nc.scalar.activation(
                            p[:, bass.ts(m, 1024)],
                            sc[m],
                            Act.Exp,
                            bias=negmx,
                            scale=1.0,
                            accum_out=zp[:, m : m + 1],
                        )
                    z = stats.tile([128, 1], f32, tag="z")
                    if nsc == 1:
                        nc.vector.tensor_copy(z, zp)
                    else:
                        nc.vector.tensor_add(z, zp[:, 0:1], zp[:, 1:2])
                    rz = stats.tile([128, 1], f32, tag="rz")
                    nc.vector.reciprocal(rz, z)
                    rzs.append(rz)

                    # ---- dropout mask (exact PRNG, prescaled {0,1.25}) ----
                    mk = masks.tile([128, sk], f16, tag="mk")
                    nc.sync.dma_start(out=mk, in_=p_mask[h, qs, :])
                    pm = probs.tile([128, sk], f16, tag="pm")
                    nc.vector.tensor_mul(pm, p, mk)

                    # ---- transpose masked probs: 128x128 blocks -> pmt ----
                    for c in range(nkc // 8):
                        tp = tpps.tile([128, 1024], f16, tag="tp")
                        for bb in range(8):
                            j = c * 8 + bb
                            nc.tensor.transpose(
                                tp[:, bass.ts(bb, 128)],
                                pm[:, bass.ts(j, 128)],
                                id16,
                            )
                        src = tp.rearrange("p (b q) -> p b q", b=8)
                        dst = pmt3[:, c * 8 : (c + 1) * 8, bass.ts(tt, 128)]
                        nc.scalar.copy(dst, src)

                # ---- PV: out.T[d, q] accumulated over k chunks ----
                pv = scps.tile([64, 512], f32, tag="sc")
                for j in range(nkc):
                    nc.tensor.matmul(
                        pv,
                        lhsT=pvw[:, bass.ts(j, 64)],
                        rhs=pmt3[:, j, :],
                        start=(j == 0),
                        stop=(j == nkc - 1),
                    )
                pvs = outs.tile([64, 512], f32, tag="pvs")
                nc.vector.tensor_copy(pvs, pv)

                # ---- transpose back to [q, d], normalize by 1/sum, store ----
                for tt in range(4):
                    t = g * 4 + tt
                    ot = scps.tile([128, 64], f32, tag="sc")
                    nc.tensor.transpose(ot, pvs[:, bass.ts(tt, 128)], id32)
                    os_ = outs.tile([128, 64], f32, tag="os")
                    nc.vector.tensor_scalar_mul(os_, ot, rzs[tt])
                    nc.sync.dma_start(out=p_out[h, bass.ts(t, 128), :], in_=os_)

    if fix_waits:
        _split_excess_waits(nc, mybir)
    return nc


def _host_inputs(x1, x2, n_heads_total=BH, sq=S, sk=S):
    """Prepare all DRAM tensors in SBUF-friendly layouts, fp32 -> fp16 hi/lo."""
    nkc = sk // 128
    x1s = (x1.astype(np.float32) * SIDE).reshape(n_heads_total, sq, D)
    x2s = (x2.astype(np.float32) * SIDE).reshape(n_heads_total, sk, D)

    x1t = np.ascontiguousarray(x1s.transpose(0, 2, 1))  # [BH, 64, SQ]
    x2t = np.ascontiguousarray(x2s.transpose(0, 2, 1))  # [BH, 64, SK]

    x1t_hi, x1t_lo = _hi(x1t), _lo(x1t)
    x2t_hi, x2t_lo = _hi(x2t), _lo(x2t)

    qk_lhs1 = np.concatenate([x1t_hi, x1t_lo], axis=1)  # [BH,128,SQ]
    qk_lhs2 = np.concatenate([x1t_hi, x1t_hi], axis=1)  # [BH,128,SQ]
    qk_rhs1 = np.concatenate([x2t_hi, x2t_hi], axis=1)  # [BH,128,SK]
    qk_rhs2 = np.concatenate(
        [x2t_lo[:, :, : sk // 2], x2t_lo[:, :, sk // 2 :]], axis=1
    )  # [BH,128,SK/2]

    # PV weights: unscaled x2, chunked [BH, 128, nkc*64]
    x2n = x2.astype(np.float32).reshape(n_heads_total, nkc, 128, D)
    pv_w = np.ascontiguousarray(x2n.transpose(0, 2, 1, 3)).reshape(
        n_heads_total, 128, nkc * D
    )
    pv_w = pv_w.astype(np.float16)
    return qk_lhs1, qk_lhs2, qk_rhs1, qk_rhs2, pv_w


def kernel(x1: np.ndarray, x2: np.ndarray) -> np.ndarray:
    from concourse.bass_utils import run_bass_kernel_spmd

    x1 = np.asarray(x1)
    x2 = np.asarray(x2)
    qk_lhs1, qk_lhs2, qk_rhs1, qk_rhs2, pv_w = _host_inputs(x1, x2)
    mask = _mask_f16()

    nc = _program()
    core_ids = list(range(N_CORES))
    in_maps = []
    for c in core_ids:
        sl = slice(c * HPC, (c + 1) * HPC)
        in_maps.append(
            {
                "qk_lhs1": qk_lhs1[sl],
                "qk_lhs2": qk_lhs2[sl],
                "qk_rhs1": qk_rhs1[sl],
                "qk_rhs2": qk_rhs2[sl],
                "pv_w": pv_w[sl],
                "mask": mask[sl],
            }
        )
    res = run_bass_kernel_spmd(nc, in_maps, core_ids)
    out = np.concatenate([r["out"] for r in res.results], axis=0)  # [BH,S,64]
    return out.reshape(B, H, S, D).astype(np.float32)
